# revision 48
# baseline (speedup 1.0000x reference)
"""Multi-head attention (b=4, n=2048, dim=1024, 16 heads x 64) on 8 Trainium2
NeuronCores.

Sharding: data-parallel over batch (4) x tensor-parallel over head-groups (2).
Each core gets one batch element and 8 heads; the host sums the two
head-group partials per batch element and adds b_out.

Per-core pipeline (v2 — bf16 data path, no PE transposes):
  x is cast fp32->bf16 in DRAM by SWDGE (gpsimd) DMAs, then xbar-transposed
  (dma_start_transpose, 14 ns/tile) straight into SBUF as [dim, n] blocks.
  The QKV projection emits qT/kT strips ([inner, n], bf16, SBUF-resident)
  and v in natural [n, inner] layout augmented with a ones column per head
  (the PV matmul then also emits the softmax denominator).

  Attention (i-block ib outer, heads inner, j-tiles innermost):
    S^T j-tile = matmul(lhsT=kT_h j-tile, rhs=qT_h i-block)  [j, i] scores
    exp on ScalarE (1/sqrt(dh) folded into the activation scale) -> bf16
    PV with P^T stationary and v_aug moving: out O[i, d] natural layout,
    65-wide moving side -> 65 cycles/matmul instead of 512 (the old
    O^T = v_aug^T @ P^T orientation paid full i-width per j-tile).
    Tail per (head, i-block): reciprocal of the denominator column
    (per-partition scalar) + one broadcast multiply -> O_big (bf16).
  O_big is staged to DRAM and xbar-transposed into oT strips; the output
  projection y = O @ w_out runs from those strips (PSUM slots borrowed
  from the po pools mid-run, psS pool at the tail).

  Emission interleaves projection units with attention steps 1:1 so the
  PE keeps busy while ScalarE (the near-critical engine) grinds exps;
  gating tracks which qT/kT/v chunks each step needs.
"""

from contextlib import ExitStack

import numpy as np

import concourse.mybir as mybir
import concourse.tile as tile
from concourse import bacc, bass_utils

F32 = mybir.dt.float32
BF16 = mybir.dt.bfloat16
FR = mybir.dt.float32r
AF = mybir.ActivationFunctionType

# Full-problem constants (hardcoded per the harness contract).
B_FULL, N_FULL, DIM_FULL = 4, 2048, 1024
HEADS_FULL, DH = 16, 64
N_CORES = 8
GROUPS = 2                       # head-group (tensor-parallel) factor
HPC = HEADS_FULL // GROUPS       # heads per core = 8
INNER_PC = HPC * DH              # per-core inner dim = 512


def ts(i, size):
    return slice(i * size, (i + 1) * size)


def emit_core_kernel(nc, tc, x, wqkv, wout, y, *, n, dim, hpc, dh, ib=1024,
                     pend_depth=2):
    inner = hpc * dh
    dh1 = dh + 1
    KC = dim // 128          # contraction chunks for the qkv projection
    SQ = inner // 128        # 128-row strips of the per-core inner dim
    JT = n // 128            # key/value j-tiles == x row-blocks
    ib = min(ib, n)
    n_ibx = n // ib
    itpb = ib // 128         # i-tiles per i-block
    FC = min(512, dim)       # projection free-dim chunk
    assert n % 512 == 0 and dim % 128 == 0 and inner % 128 == 0
    assert ib % 512 == 0 and n % ib == 0
    scale = float(1.0 / np.sqrt(dh))

    q_done = set()           # (s, isub) emitted
    k_done = set()           # (s, jsub)
    v_done = set()           # jsub

    stack = ExitStack()
    with stack:
        persist = stack.enter_context(tc.tile_pool(name="persist", bufs=1))
        dram_pool = stack.enter_context(
            tc.tile_pool(name="dram", bufs=1, space="DRAM"))
        HBD = min(1024, n)
        xbf_dram = dram_pool.tile([n, dim], BF16, name="xbf")
        o_dram2 = [[dram_pool.tile([ib, 128], BF16, name="odr",
                                   tag=f"od{bx}_{s}") for s in range(SQ)]
                   for bx in range(n_ibx)]

        # Persistent SBUF tensors.
        qT = [persist.tile([128, n], BF16, name="qTs", tag=f"qT{s}")
              for s in range(SQ)]
        kT = [persist.tile([128, n], BF16, name="kTs", tag=f"kT{s}")
              for s in range(SQ)]
        v_sb = [persist.tile([128, hpc * dh1], BF16, name="vts",
                             tag=f"v{jt}") for jt in range(JT)]
        O_big = persist.tile([128, JT * inner], BF16, name="O_big")
        oT = [[persist.tile([128, ib], BF16, name="oTs", tag=f"oT{s}_{bx}")
               for bx in range(n_ibx)] for s in range(SQ)]
        wout_sb = []

        XQ = min(512, n)

        # ---- phase B pools (created first so PSUM layout is stable) ----
        psS_pool = stack.enter_context(
            tc.tile_pool(name="b_psS", bufs=2, space="PSUM"))
        po_pools = [stack.enter_context(
            tc.tile_pool(name="b_po1", bufs=1, space="PSUM"))]
        pexp_pool = stack.enter_context(tc.tile_pool(name="b_pexp", bufs=12))
        tail_pool = stack.enter_context(tc.tile_pool(name="b_tail", bufs=4))
        late_pools = {}

        # ---- phase A pools ----
        actx = ExitStack()
        w_pool = actx.enter_context(tc.tile_pool(name="a_w", bufs=1))
        xts_pool = actx.enter_context(tc.tile_pool(name="a_xts", bufs=1))
        psA_pool = actx.enter_context(
            tc.tile_pool(name="a_ps", bufs=2, space="PSUM"))

        # Weights arrive as bf16 via SWDGE cast-DMAs (gpsimd): half the
        # bytes of fp32 loads and SBUF. Strip-0 q/k columns come first so
        # heads 0-1 unblock early; w_v early too (PV trails by ~2 steps).
        wr = inner - 128
        wqkv_r = wqkv.rearrange("(kc p) c -> p kc c", p=128)
        wq_s0 = w_pool.tile([128, KC * 128], BF16, name="wqs0", tag="wqs0")
        wk_s0 = w_pool.tile([128, KC * 128], BF16, name="wks0", tag="wks0")
        wv_sb = w_pool.tile([128, KC * inner], BF16, name="wv", tag="wv")
        if wr:
            wq_r = w_pool.tile([128, KC * wr], BF16, name="wqr", tag="wqr")
            wk_r = w_pool.tile([128, KC * wr], BF16, name="wkr", tag="wkr")
        XQ = min(512, n)
        NXQ = n // XQ
        NH = n // HBD
        QPH = HBD // XQ          # x cast quarters per half tile

        def cast_half(hh):
            nc.gpsimd.dma_start(
                xbf_dram[ts(hh, HBD), :], x[ts(hh, HBD), :])

        # Pool (SWDGE) stream: the first half of x and the v_aug ones
        # columns. The remaining halves are emitted later (cast_rest unit)
        # behind a fence on the first k copy, so those transfers cannot cut
        # ahead of the half-0 transposes in the DMA FIFO.
        cast_half(0)
        for jt in range(JT):
            nc.gpsimd.memset(
                v_sb[jt].rearrange("p (h c) -> p h c", c=dh1)[:, :, dh:dh1],
                1.0)

        # SP (HWDGE) stream: strip-0 q/k columns, first transposes, then v
        # weights, then the remaining q/k columns and transposes.
        nc.sync.dma_start(
            wq_s0.rearrange("p (kc c) -> p kc c", c=128),
            wqkv_r[:, :, 0:128])
        nc.sync.dma_start(
            wk_s0.rearrange("p (kc c) -> p kc c", c=128),
            wqkv_r[:, :, inner:inner + 128])

        def w_qk(which, kc, s):
            if s == 0:
                t = wq_s0 if which == "q" else wk_s0
                return t[:, ts(kc, 128)]
            t = wq_r if which == "q" else wk_r
            return t[:, kc * wr + (s - 1) * 128: kc * wr + s * 128]

        def w_v(kc):
            return wv_sb[:, ts(kc, inner)]

        # xT half-strips, one xbar transpose each (contiguous destination)
        xh_t = [[xts_pool.tile([128, HBD], BF16, name="xts",
                               tag=f"xth{kc}_{hh}") for hh in range(NH)]
                for kc in range(KC)]
        for kc in range(KC):
            nc.sync.dma_start_transpose(
                xh_t[kc][0], xbf_dram[ts(0, HBD), ts(kc, 128)])
        nc.sync.dma_start(
            wv_sb.rearrange("p (kc c) -> p kc c", c=inner),
            wqkv_r[:, :, 2 * inner:3 * inner])
        if wr:
            nc.sync.dma_start(
                wq_r.rearrange("p (kc c) -> p kc c", c=wr),
                wqkv_r[:, :, 128:inner])
            nc.sync.dma_start(
                wk_r.rearrange("p (kc c) -> p kc c", c=wr),
                wqkv_r[:, :, inner + 128:2 * inner])

        def xts(kc, rb):
            hh, r = divmod(rb * 128, HBD)
            return xh_t[kc][hh][:, r:r + 128]

        # ---- phase A units ----
        def qu(s, isub):
            def f():
                ps = psA_pool.tile([128, 128], F32, name="psA")
                for kc in range(KC):
                    nc.tensor.matmul(ps, w_qk("q", kc, s),
                                     xts(kc, isub), start=(kc == 0),
                                     stop=(kc == KC - 1))
                nc.vector.tensor_copy(qT[s][:, ts(isub, 128)], ps)
                q_done.add((s, isub))
            return f

        def ku(s, jsub):
            def f():
                ps = psA_pool.tile([128, 128], F32, name="psA")
                for kc in range(KC):
                    nc.tensor.matmul(ps, w_qk("k", kc, s),
                                     xts(kc, jsub), start=(kc == 0),
                                     stop=(kc == KC - 1))
                nc.vector.tensor_copy(kT[s][:, ts(jsub, 128)], ps)
                k_done.add((s, jsub))
            return f

        def vu(jsub, half):
            # half 0: heads [0, hpc/2) columns; half 1: the rest
            h0 = half * (hpc // 2)
            h1 = (half + 1) * (hpc // 2)
            c0, c1 = h0 * dh, h1 * dh
            def f():
                ps = psA_pool.tile([128, c1 - c0], F32, name="psA")
                for kc in range(KC):
                    nc.tensor.matmul(ps, xts(kc, jsub),
                                     w_v(kc)[:, c0:c1],
                                     start=(kc == 0), stop=(kc == KC - 1))
                nc.vector.tensor_copy(
                    v_sb[jsub].rearrange("p (h c) -> p h c",
                                         c=dh1)[:, h0:h1, 0:dh],
                    ps.rearrange("p (h c) -> p h c", c=dh))
                v_done.add((jsub, half))
            return f

        # unit order: q strip 0 for i-block 0 first (unblocks heads 0-1),
        # then k strip 0 + v interleaved, then remaining strips, then the
        # later i-blocks' q columns.
        def cast_rest():
            # the half-1 cast writes the same DRAM tile the half-0
            # transposes read, so its WAR dependency keeps it from cutting
            # ahead of them in the DMA queue
            for hh in range(1, NH):
                cast_half(hh)
            for hh in range(1, NH):
                for kc in range(KC):
                    nc.sync.dma_start_transpose(
                        xh_t[kc][hh], xbf_dram[ts(hh, HBD), ts(kc, 128)])

        # (emit-fn, PE-weight, earliest step): the min step holds a unit
        # back until its DMA inputs have plausibly landed, so an in-order
        # PE never parks on a far-future transfer.
        units = []
        ib0 = range(itpb)
        units += [(cast_rest, 0.05, 0)]
        units += [(qu(0, i), 0.43, 0) for i in ib0]
        units += [(ku(0, j), 0.43, 0) for j in range(min(8, JT))]
        units += [(ku(0, j), 0.43, 7) for j in range(8, JT)]
        units += [(vu(j, 0), 0.85, 9) for j in range(JT)]
        for s in range(1, SQ):
            units += [(qu(s, i), 0.43, 14 + 4 * s) for i in ib0]
            units += [(ku(s, j), 0.43, 14 + 4 * s) for j in range(JT)]
        units += [(vu(j, 1), 0.85, 30) for j in range(JT)]
        for bx in range(1, n_ibx):
            for s in range(SQ):
                units += [(qu(s, i), 0.43, 38)
                          for i in range(bx * itpb, (bx + 1) * itpb)]

        # ---- phase B ----
        steps = [(bx, h, jt) for bx in range(n_ibx) for h in range(hpc)
                 for jt in range(JT)]

        def gate_ok(k):
            # S needs only q and k; v readiness gates the PV pops instead
            bx, h, jt = steps[k]
            s = (h * dh) // 128
            return (all((s, i) in q_done
                        for i in range(bx * itpb, (bx + 1) * itpb))
                    and (s, jt) in k_done)

        po_alt = [0]

        def alloc_po():
            pool = po_pools[po_alt[0] % len(po_pools)]
            po_alt[0] += 1
            return pool.tile([128, itpb * 128], F32, name="po")

        head_state = {}
        pend = []                 # (po, pexp, jt, h, bx)
        ocnt = [0] * n_ibx
        proj_due = []
        projected = set()

        def emit_s(k):
            bx, h, jt = steps[k]
            s, r = divmod(h * dh, 128)
            psS = psS_pool.tile([128, ib], F32, name="psS")
            for c in range(ib // 512):
                nc.tensor.matmul(
                    psS[:, ts(c, 512)], kT[s][r:r + dh, ts(jt, 128)],
                    qT[s][r:r + dh, bx * ib + c * 512: bx * ib + (c + 1) * 512],
                    start=True, stop=True)
            return psS

        ysb_half = {}
        SH = max(1, SQ // 2)
        two_pass = SQ >= 2

        def wo(t):
            return wout_sb[0][:, ts(t, dim)]

        def load_wout():
            if not wout_sb:
                wo_big = late_pools["wout"].tile([128, SQ * dim], BF16,
                                                 name="wo", tag="wo")
                nc.sync.dma_start(
                    wo_big.rearrange("p (t c) -> p t c", c=dim),
                    wout.rearrange("(t p) c -> p t c", p=128))
                wout_sb.append(wo_big)

        def store_o(bx, s):
            nc.sync.dma_start(
                o_dram2[bx][s][:, :].rearrange("(t p) c -> p t c", p=128),
                O_big.rearrange("p (t c) -> p t c", c=inner)[
                    :, ts(bx, itpb), ts(s, 128)])

        def emit_drain(po, h, bx):
            pv = po.rearrange("p (t c) -> p t c", c=128)
            den = pv[:, :, dh:dh1].rearrange("p t c -> p (t c)")
            rc = tail_pool.tile([128, itpb], F32, name="rc")
            nc.vector.reciprocal(rc, den)
            out = O_big.rearrange("p (t u) -> p t u", u=inner)[
                :, ts(bx, itpb), ts(h, dh)]
            nc.vector.tensor_mul(
                out, pv[:, :, 0:dh],
                rc.rearrange("p t -> p t ()").broadcast_to([128, itpb, dh]))
            ocnt[bx] += 1
            if ocnt[bx] % 2 == 0:
                s = ocnt[bx] // 2 - 1
                store_o(bx, s)
                nc.sync.dma_start_transpose(oT[s][bx], o_dram2[bx][s][:, :])
            its = range(bx * itpb, (bx + 1) * itpb)
            if two_pass and ocnt[bx] == 2 * SH:
                proj_due.extend((bx, it, 1) for it in its)
            if ocnt[bx] == hpc:
                proj_due.extend(
                    (bx, it, 3 if two_pass else 0) for it in its)

        def emit_proj(bx, it, pss, pool, name):
            load_wout()
            trange = (range(SH) if pss == 1 else
                      range(SH, SQ) if pss == 3 else range(SQ))
            psY = pool.tile([128, dim], F32, name=name)
            lo = (it - bx * itpb) * 128
            tl = list(trange)
            for i2, t in enumerate(tl):
                for c in range(dim // FC):
                    nc.tensor.matmul(
                        psY[:, ts(c, FC)], oT[t][bx][:, lo:lo + 128],
                        wo(t)[:, ts(c, FC)],
                        start=(i2 == 0), stop=(i2 == len(tl) - 1))
            if pss == 1:
                yb = late_pools["ybf"].tile([128, dim], BF16, name="ybf")
                nc.vector.tensor_copy(yb, psY)
                ysb_half[it] = yb
                return
            yf = late_pools["y"].tile([128, dim], BF16, name="ysb")
            if pss == 3:
                nc.vector.tensor_add(yf, psY, ysb_half.pop(it))
            else:
                nc.vector.tensor_copy(yf, psY)
            nc.sync.dma_start(y[ts(it, 128), :], yf)
            projected.add((bx, it))

        def wout_ok():
            return "wout" in late_pools

        def pop_pend():
            pexp, jt, h, bx = pend.pop(0)
            gi = (bx, h)
            if jt == 0:
                head_state[gi] = alloc_po()
            po = head_state[gi]
            vcol = slice(h * dh1, (h + 1) * dh1)
            # po packs 4 x 512B i-tile slices per 2KB psum zero region: one
            # start (zeroes the region) and one stop per bank, not per slice
            spb = 4
            for t in range(itpb):
                nc.tensor.matmul(
                    po[:, t * 128:t * 128 + dh1], pexp[:, ts(t, 128)],
                    v_sb[jt][:, vcol],
                    start=(jt == 0 and t % spb == 0),
                    stop=(jt == JT - 1
                          and (t % spb == spb - 1 or t == itpb - 1)))
            if jt == JT - 1:
                emit_drain(po, h, bx)
                head_state.pop(gi, None)
                # weave deferred projections into the po slot that just
                # drained (same pool: sequential tenancy after the drain)
                if wout_ok():
                    pool = po_pools[po_alt[0] % len(po_pools)]
                    po_alt[0] += 1
                    for _ in range(6):
                        if proj_due:
                            emit_proj(*proj_due.pop(0), pool=pool, name="po")

        s_ahead = None

        def emit_bstep(k):
            nonlocal s_ahead
            if s_ahead is None:
                s_ahead = emit_s(k)
            psS = s_ahead
            bx, h, jt = steps[k]
            nx = k + 1
            if nx < len(steps) and gate_ok(nx):
                s_ahead = emit_s(nx)
            else:
                s_ahead = None
            pexp = pexp_pool.tile([128, ib], BF16, name="pexp")
            nc.scalar.activation(pexp, psS, AF.Exp, scale=scale)
            pend.append((pexp, jt, h, bx))
            while (len(pend) > pend_depth
                   and (pend[0][1], pend[0][2] // (hpc // 2)) in v_done):
                pop_pend()

        # ---- drive ----
        bi = 0
        ui = 0
        credit = [0.0]

        def emit_unit():
            nonlocal ui
            f, w, _ms = units[ui]
            f()
            credit[0] -= w
            ui += 1
            if ui == len(units):
                actx.close()
                po_pools.append(stack.enter_context(
                    tc.tile_pool(name="b_po2", bufs=1, space="PSUM")))
                late_pools["wout"] = stack.enter_context(
                    tc.tile_pool(name="c_w", bufs=1))
                late_pools["ybf"] = stack.enter_context(
                    tc.tile_pool(name="c_ybf", bufs=min(JT, 10)))
                late_pools["y"] = stack.enter_context(
                    tc.tile_pool(name="c_y", bufs=2))

        PACE = 0.40
        while bi < len(steps):
            if gate_ok(bi):
                # clear the next step's gate first so the S-lead in
                # emit_bstep succeeds (ScalarE never waits on a late S)
                while (ui < len(units) and bi + 1 < len(steps)
                       and not gate_ok(bi + 1)):
                    emit_unit()
                emit_bstep(bi)
                bi += 1
                credit[0] += PACE
                while (ui < len(units) and credit[0] > 0
                       and bi >= units[ui][2]):
                    emit_unit()
                # if PV pops are blocked on v units, nudge them through so
                # the pexp pool never backs up into ScalarE
                k2 = 0
                while (len(pend) > pend_depth + 2 and ui < len(units)
                       and bi >= units[ui][2] and k2 < 2):
                    emit_unit()
                    k2 += 1
            elif ui < len(units):
                emit_unit()
            else:
                raise RuntimeError("B blocked with no A units left")
        while ui < len(units):
            emit_unit()
        while pend:
            pop_pend()
        # remaining projections on the freed psS pool (4 banks deep)
        fl = 0
        while proj_due:
            pool, name = ([(psS_pool, "psS")] + [(p, "po") for p in po_pools]
                          )[fl % (1 + len(po_pools))]
            fl += 1
            emit_proj(*proj_due.pop(0), pool=pool, name=name)


_BUILD_CACHE = {}


def build_nc(n=N_FULL, dim=DIM_FULL, hpc=HPC, dh=DH, mm_dt=None, ib=1024,
             pend_depth=2, **_ignored):
    key = (n, dim, hpc, dh, ib, pend_depth)
    if key in _BUILD_CACHE:
        return _BUILD_CACHE[key]
    inner = hpc * dh
    nc = bacc.Bacc("TRN2", target_bir_lowering=False, debug=False)
    x = nc.dram_tensor("x", [n, dim], F32, kind="ExternalInput").ap()
    wqkv = nc.dram_tensor("w_qkv", [dim, 3 * inner], BF16,
                          kind="ExternalInput").ap()
    wout = nc.dram_tensor("w_out", [inner, dim], BF16,
                          kind="ExternalInput").ap()
    y = nc.dram_tensor("y", [n, dim], BF16,
                       kind="ExternalOutput").ap()
    with tile.TileContext(nc) as tc:
        with nc.allow_low_precision(
                reason="bf16 attention core; PSUM accumulation stays fp32"):
            emit_core_kernel(nc, tc, x, wqkv, wout, y, n=n, dim=dim,
                             hpc=hpc, dh=dh, ib=ib, pend_depth=pend_depth)
    nc.compile()
    _BUILD_CACHE[key] = nc
    return nc


def make_in_maps(x, w_qkv, w_out):
    """Shard full inputs into the 8 per-core input maps (weights in bf16)."""
    import ml_dtypes
    x = np.asarray(x, dtype=np.float32)
    w_qkv = np.asarray(w_qkv, dtype=np.float32).astype(ml_dtypes.bfloat16)
    w_out = np.asarray(w_out, dtype=np.float32).astype(ml_dtypes.bfloat16)
    qk_off = HEADS_FULL * DH          # 1024: start of K block in w_qkv
    in_maps = []
    for c in range(N_CORES):
        b, g = divmod(c, GROUPS)
        cols = ts(g, INNER_PC)
        wq = w_qkv[:, cols]
        wk = w_qkv[:, qk_off + g * INNER_PC: qk_off + (g + 1) * INNER_PC]
        wv = w_qkv[:, 2 * qk_off + g * INNER_PC: 2 * qk_off + (g + 1) * INNER_PC]
        in_maps.append({
            "x": np.ascontiguousarray(x[b]),
            "w_qkv": np.ascontiguousarray(np.concatenate([wq, wk, wv], axis=1)),
            "w_out": np.ascontiguousarray(w_out[cols, :]),
        })
    return in_maps


def kernel(x, w_qkv, w_out, b_out, trace=False):
    b_out = np.asarray(b_out, dtype=np.float32)
    nc = build_nc()
    in_maps = make_in_maps(x, w_qkv, w_out)
    res = bass_utils.run_bass_kernel_spmd(
        nc, in_maps, core_ids=list(range(N_CORES)), trace=trace)
    ys = [np.asarray(r["y"], dtype=np.float32) for r in res.results]
    out = np.empty((B_FULL, N_FULL, DIM_FULL), dtype=np.float32)
    for b in range(B_FULL):
        out[b] = ys[GROUPS * b] + ys[GROUPS * b + 1] + b_out[None, :]
    if trace:
        kernel.last_result = res
    return out
